# revision 32
# baseline (speedup 1.0000x reference)
"""Trainium2 Bass kernel for nn_CrossAttention (B=8, S1=S2=2048, D=512, single head).

Sharding: batch dim B=8 across the 8 NeuronCores (data parallel). Each core runs
the full cross-attention for one batch element:
    q = RoPE(h1 @ Wq.T + bq); k = RoPE(h2 @ Wk.T + bk); v = h2 @ Wv.T + bv
    out = softmax(q k^T / sqrt(D)) v @ Wo.T + bo

Design notes:
  - All matmuls in bf16 (fp32 PSUM accumulation): rel_l2 vs fp32 reference ~6e-3.
  - Scores are computed TRANSPOSED (S^T[k,q]) so the probability matrix feeds the
    PV matmul directly as the moving operand - no P transposes.
  - Softmax skips max-subtraction (energies are ~N(0,1), |e| < 8, exp is safe in
    fp32) so no partition-dim max is needed.
  - Denominators WITHOUT any PE ones-matmuls: the 16 exp'd pt tiles per q tile
    are pairwise-summed on the DVE (bf16 tree, fp32 final level), the [128,512]
    accumulator is PE-transposed in 4 chunks, free-axis-reduced on DVE to
    [128q,1] columns, and the reciprocal runs on [128,4] (per-lane cheap).
    Normalization lands after the Wo projection, fused into the output STT.
  - The prologue is DMA-bandwidth-bound (the HBM stack is shared with the pair
    core; ~150-320 KB/us per core during the all-cores burst), so q-tile 0's
    attention blocks are INTERLEAVED into phase A: as soon as k/v/q slice s is
    projected, key blocks 4s..4s+3 run - the PE fills DMA waits with ready
    attention work instead of idling.
  - HAM warm-up: a few junk matmuls bridge engine-start -> first data chunk so
    the PE clock-gate reaches 2.4 GHz before real work.
  - PV matmuls lag the score matmuls by TWO key blocks so the ACT exp never
    head-of-line blocks the PE.
  - Output is written bf16 (halves write traffic; host upcasts to fp32).
  - h1/h2/weights/tables are pre-transposed+packed on host (bf16); DMAs are
    emitted in exact consumption order, with the first-matmul tensors (wk, h2
    slice 0) split into ec-quarters so chunk-0 matmuls start ~4us earlier.
  - PSUM (8 banks): st 2x1 (shared with denominator transposes) + ot 1x2 +
    pp 2x1 (projection halves) + work 2x1 (v-projection copies / fin groups).
"""

import math
import sys

import numpy as np

for _p in ("/opt/trn_rl_repo",):
    if _p not in sys.path:
        sys.path.insert(0, _p)

import ml_dtypes

BF16 = ml_dtypes.bfloat16

S = 2048
D = 512
P = 128
B = 8
NB = S // P      # 16 key blocks of 128
DC = D // P      # 4 d-chunks of 128
EC = D // P      # 4 e-chunks (contraction for projections)
QW = 512         # tile width (free dim per matmul)
QT = S // QW     # 4 q tiles
SB = QW // P     # 4 s-blocks per q tile
NS = S // QW     # 4 s-slices for the prologue
SCALE = 1.0 / math.sqrt(D)
NWARM = 6        # junk matmuls bridging engine start -> first data chunk

_compiled = None


def _build():
    import concourse.bass as bass  # noqa: F401
    import concourse.mybir as mybir
    import concourse.tile as tile
    from concourse import bacc

    f32 = mybir.dt.float32
    bf16 = mybir.dt.bfloat16
    Alu = mybir.AluOpType
    Act = mybir.ActivationFunctionType
    AxX = mybir.AxisListType.X

    nc = bacc.Bacc("TRN2", target_bir_lowering=False, debug=False, num_devices=B)

    h1t_d = nc.dram_tensor("h1t", [P, NS, EC, QW], bf16, kind="ExternalInput").ap()
    h2t_d = nc.dram_tensor("h2t", [P, NS, EC, QW], bf16, kind="ExternalInput").ap()
    w_dram = {
        name: nc.dram_tensor(f"{name}_t", [P, EC * D], bf16, kind="ExternalInput").ap()
        for name in ("wq", "wk", "wv", "wo")
    }
    cos_t = nc.dram_tensor("cos_t", [P, 2 * S], bf16, kind="ExternalInput").ap()
    sin_t = nc.dram_tensor("sin_t", [P, 2 * S], bf16, kind="ExternalInput").ap()
    bq_c = nc.dram_tensor("bq_c", [P, DC], f32, kind="ExternalInput").ap()
    bk_c = nc.dram_tensor("bk_c", [P, DC], f32, kind="ExternalInput").ap()
    # bo_b holds bo_eff = bo + Wo @ bv (bv folded through the value path on host)
    bo_b = nc.dram_tensor("bo_b", [P, D], f32, kind="ExternalInput").ap()
    ident_d = nc.dram_tensor("ident", [P, P], f32, kind="ExternalInput").ap()
    out = nc.dram_tensor("out", [S, D], bf16, kind="ExternalOutput").ap()

    with tile.TileContext(nc) as tc:
        from contextlib import ExitStack

        with ExitStack() as ctx:
            singles = ctx.enter_context(tc.tile_pool(name="singles", bufs=1))

            # --- HAM warm-up: PE is idle until the first DMA chunks land;
            # junk matmuls on a zeroed tile keep it busy so the clock-gate
            # starts releasing before real work arrives.
            warm_sb = singles.tile([P, QW], bf16, tag="warm")
            nc.vector.memset(warm_sb, 0.0)

            def load_w(name, eng=None):
                t = singles.tile([P, EC, D], bf16, tag=f"w_{name}")
                (eng or nc.sync).dma_start(
                    out=t, in_=w_dram[name].rearrange("p (c d) -> p c d", d=D)
                )
                return t

            # --- persistent tiles ------------------------------------------
            w_sb = {}
            kt_p = [
                singles.tile([P, DC, QW], bf16, tag=f"kt{i}", name=f"kt{i}")
                for i in range(NS)
            ]
            qt_p = [
                singles.tile([P, DC, QW], bf16, tag=f"qt{i}", name=f"qt{i}")
                for i in range(NS)
            ]
            v_p = [
                singles.tile([P, SB, QW], bf16, tag=f"v{i}", name=f"v{i}")
                for i in range(NS)
            ]
            cos_sb = singles.tile([P, 2, S], bf16, tag="cos")
            sin_sb = singles.tile([P, 2, S], bf16, tag="sin")
            h1s = [
                singles.tile([P, EC, QW], bf16, tag=f"h1s{i}", name=f"h1s{i}")
                for i in range(NS)
            ]
            h2s = [
                singles.tile([P, EC, QW], bf16, tag=f"h2s{i}", name=f"h2s{i}")
                for i in range(NS)
            ]

            # --- pools (SBUF + the unified 8-bank PSUM layout) --------------
            ptmp = ctx.enter_context(tc.tile_pool(name="ptmp", bufs=3))
            ptp = ctx.enter_context(tc.tile_pool(name="ptpool", bufs=NB + 2))
            otp = ctx.enter_context(tc.tile_pool(name="otsb", bufs=2))
            outp = ctx.enter_context(tc.tile_pool(name="outst", bufs=3))
            acc1p = ctx.enter_context(tc.tile_pool(name="acc1p", bufs=8))
            acc2p = ctx.enter_context(tc.tile_pool(name="acc2p", bufs=4))
            acc3p = ctx.enter_context(tc.tile_pool(name="acc3p", bufs=2))
            accfp = ctx.enter_context(tc.tile_pool(name="accfp", bufs=2))
            # one 4-deep single-bank ring is SHARED by projection pp halves,
            # score tiles, and the denominator transposes: deep pipelining
            # for whichever of them is active (8 banks: mm 4 + ot 2 + work 2)
            ps_mm = ctx.enter_context(tc.tile_pool(name="psum_mm", bufs=4, space="PSUM"))
            ps_ot = ctx.enter_context(tc.tile_pool(name="psum_ot", bufs=1, space="PSUM"))
            ps_wk = ctx.enter_context(tc.tile_pool(name="psum_wk", bufs=2, space="PSUM"))

            def junk_mms(n):
                # clock-gate/idle filler: matmuls on the zeroed tile into a
                # work-ring bank nothing reads
                if n <= 0:
                    return
                wps = ps_wk.tile([P, QW], f32, tag="work", name="wps")
                for _ in range(n):
                    nc.tensor.matmul(
                        wps, lhsT=warm_sb[:, 0:P], rhs=warm_sb, start=True, stop=True
                    )

            def project_rope(ht, wname, b_sb, dst, s2, junk=0):
                # dst[:, dc, :] = RoPE(W @ h^T + b) for columns of slice s2.
                # The 4 chains (2 pairs x 2 halves) advance EC-MAJOR so each
                # arriving ec chunk of ht/W is consumed by 4 matmuls at once
                # (matches the chunk-split DMA pacing); `junk` filler MMs per
                # ec gap keep the PE busy/warm when the chunks are the gate.
                sl = slice(s2 * QW, (s2 + 1) * QW)
                chains = [(0, 0), (0, 2), (1, 1), (1, 3)]  # (pair, dc)
                pps = [
                    ps_mm.tile([P, QW], f32, tag="mm", name=f"pp{ci}")
                    for ci in range(4)
                ]
                for ec in range(EC):
                    for ci, (pair, dc) in enumerate(chains):
                        nc.tensor.matmul(
                            pps[ci],
                            lhsT=w_sb[wname][:, ec, dc * P : (dc + 1) * P],
                            rhs=ht[:, ec, :],
                            start=(ec == 0),
                            stop=(ec == EC - 1),
                        )
                    if ec < EC - 1:
                        junk_mms(junk)
                for pair in range(2):
                    pp0, pp1 = pps[2 * pair], pps[2 * pair + 1]
                    dc0, dc2 = pair, pair + 2
                    # rope: dst[dc0] = x0*cos - x2*sin ; dst[dc2] = x2*cos + x0*sin
                    # (bias folds into the STT's scalar add; combines on GpSimd)
                    cps = cos_sb[:, pair, sl]
                    sps = sin_sb[:, pair, sl]
                    t0 = ptmp.tile([P, QW], f32, tag="rope0")
                    nc.vector.scalar_tensor_tensor(
                        t0, in0=pp0, scalar=b_sb[:, dc0 : dc0 + 1], in1=cps,
                        op0=Alu.add, op1=Alu.mult,
                    )
                    t3 = ptmp.tile([P, QW], f32, tag="rope1")
                    nc.vector.scalar_tensor_tensor(
                        t3, in0=pp0, scalar=b_sb[:, dc0 : dc0 + 1], in1=sps,
                        op0=Alu.add, op1=Alu.mult,
                    )
                    t1 = ptmp.tile([P, QW], f32, tag="rope2")
                    nc.vector.scalar_tensor_tensor(
                        t1, in0=pp1, scalar=b_sb[:, dc2 : dc2 + 1], in1=sps,
                        op0=Alu.add, op1=Alu.mult,
                    )
                    t2 = ptmp.tile([P, QW], f32, tag="rope3")
                    nc.vector.scalar_tensor_tensor(
                        t2, in0=pp1, scalar=b_sb[:, dc2 : dc2 + 1], in1=cps,
                        op0=Alu.add, op1=Alu.mult,
                    )
                    nc.gpsimd.tensor_tensor(dst[:, dc0, :], t0, t1, Alu.subtract)
                    nc.gpsimd.tensor_tensor(dst[:, dc2, :], t2, t3, Alu.add)

            def project_v(s2):
                # bv is folded into bo on host, so this is a plain PSUM->SBUF
                # cast on the ACT engine
                for j in range(SB):
                    vp = ps_wk.tile([P, QW], f32, tag="work", name="vp")
                    for ec in range(EC):
                        nc.tensor.matmul(
                            vp,
                            lhsT=h2s[s2][:, ec, j * P : (j + 1) * P],
                            rhs=w_sb["wv"][:, ec, :],
                            start=(ec == 0),
                            stop=(ec == EC - 1),
                        )
                    nc.scalar.copy(v_p[s2][:, j, :], vp)

            def emit_pv(pt, ot, kb, dcs):
                for i, dc in enumerate(dcs):
                    nc.tensor.matmul(
                        ot[:, i, :],
                        lhsT=v_p[kb // SB][:, kb % SB, dc * P : (dc + 1) * P],
                        rhs=pt,
                        start=(kb == 0),
                        stop=(kb == NB - 1),
                    )

            def emit_kb_block(qt, kb, qs, emit_lagged=True):
                # one key block of pass 1: S^T -> exp -> (lagged) PV dc 0,1,
                # with the denominator tree riding along on the DVE
                st = ps_mm.tile([P, QW], f32, tag="mm", name="st")
                for dc in range(DC):
                    nc.tensor.matmul(
                        st,
                        lhsT=kt_p[kb // SB][:, dc, (kb % SB) * P : (kb % SB + 1) * P],
                        rhs=qt_p[qt][:, dc, :],
                        start=(dc == 0),
                        stop=(dc == DC - 1),
                    )
                pt = ptp.tile([P, QW], bf16, tag="pt")
                nc.scalar.activation(pt, st, Act.Exp, scale=SCALE)
                qs["pts"].append(pt)
                if kb % 2 == 1:
                    a = acc1p.tile([P, QW], bf16, tag="acc1")
                    nc.vector.tensor_tensor(a, qs["pts"][kb - 1], qs["pts"][kb], Alu.add)
                    qs["l1"].append(a)
                if kb in (3, 7, 11):
                    b = acc2p.tile([P, QW], bf16, tag="acc2")
                    nc.vector.tensor_tensor(b, qs["l1"][-2], qs["l1"][-1], Alu.add)
                    qs["l2"].append(b)
                # the tail levels are RUNNING partials so accf (which gates
                # the denominator transposes) is ready 2 DVE ops after the
                # last exp instead of 4
                if kb == 7:
                    c = acc3p.tile([P, QW], bf16, tag="acc3")
                    nc.vector.tensor_tensor(c, qs["l2"][0], qs["l2"][1], Alu.add)
                    qs["l3"].append(c)
                if kb == 11:
                    pa = acc3p.tile([P, QW], bf16, tag="acc3")
                    nc.vector.tensor_tensor(pa, qs["l3"][0], qs["l2"][2], Alu.add)
                    qs["pa"] = pa
                if kb == 13:
                    pb = acc3p.tile([P, QW], bf16, tag="acc3")
                    nc.vector.tensor_tensor(pb, qs["pa"], qs["l1"][6], Alu.add)
                    qs["pb"] = pb
                if kb == NB - 1:
                    accf = accfp.tile([P, QW], f32, tag="accf")
                    nc.vector.tensor_tensor(accf, qs["pb"], qs["l1"][7], Alu.add)
                    qs["accf"] = accf
                if kb >= 2 and emit_lagged:
                    emit_pv(qs["pts"][kb - 2], qs["ot01"], kb - 2, (0, 1))

            def new_qstate():
                return {
                    "pts": [], "l1": [], "l2": [], "l3": [],
                    "ot01": ps_ot.tile([P, 2, QW], f32, tag="ot", name="ot01"),
                    "ot_sb": otp.tile([P, DC, QW], bf16, tag="ot_sb", name="ot_sb"),
                }

            def finish_qt(qt, qs):
                # tail PVs + ot01 copies, denominators, pass 2, fin
                pts = qs["pts"]
                ot01, ot_sb = qs["ot01"], qs["ot_sb"]
                emit_pv(pts[NB - 2], ot01, NB - 2, (0, 1))
                emit_pv(pts[NB - 1], ot01, NB - 1, (0, 1))
                # split the two copies across engines so they land in one
                # copy-latency, not two
                nc.scalar.copy(ot_sb[:, 0, :], ot01[:, 0, :])
                nc.vector.tensor_copy(out=ot_sb[:, 1, :], in_=ot01[:, 1, :])

                # denominators: 4 PE transposes (cover the ot01 copies' bank
                # turnaround), DVE reduce + reciprocal
                accf = qs["accf"]
                tpr = ps_mm.tile([P, SB, P], f32, tag="mm", name="tpr")
                for j in range(SB):
                    nc.tensor.transpose(
                        tpr[:, j, :], accf[:, j * P : (j + 1) * P], ident
                    )
                z4 = outp.tile([P, SB], f32, tag="z4")
                for j in range(SB):
                    nc.vector.tensor_reduce(
                        z4[:, j : j + 1], tpr[:, j, :], axis=AxX, op=Alu.add
                    )
                r4r = outp.tile([P, SB], f32, tag="r4r")
                nc.vector.reciprocal(r4r, z4)

                # pass 2: PV dc 2,3 from the resident pt tiles
                ot23 = ps_ot.tile([P, 2, QW], f32, tag="ot", name="ot23")
                for kb in range(NB):
                    for i, dc in enumerate((2, 3)):
                        nc.tensor.matmul(
                            ot23[:, i, :],
                            lhsT=v_p[kb // SB][:, kb % SB, dc * P : (dc + 1) * P],
                            rhs=pts[kb],
                            start=(kb == 0),
                            stop=(kb == NB - 1),
                        )
                nc.scalar.copy(ot_sb[:, 2, :], ot23[:, 0, :])
                nc.vector.tensor_copy(out=ot_sb[:, 3, :], in_=ot23[:, 1, :])

                # final projection; fused (fp * r) + bo in one DVE op, bf16 out
                for sb in range(SB):
                    fp = ps_wk.tile([P, QW], f32, tag="work", name="fpt")
                    for dc in range(DC):
                        nc.tensor.matmul(
                            fp,
                            lhsT=ot_sb[:, dc, sb * P : (sb + 1) * P],
                            rhs=w_sb["wo"][:, dc, :],
                            start=(dc == 0),
                            stop=(dc == DC - 1),
                        )
                    o_sb = outp.tile([P, D], bf16, tag="ostage")
                    nc.vector.scalar_tensor_tensor(
                        o_sb,
                        in0=fp,
                        scalar=r4r[:, sb : sb + 1],
                        in1=bo_sb,
                        op0=Alu.mult,
                        op1=Alu.add,
                    )
                    row0 = (qt * SB + sb) * P
                    nc.sync.dma_start(out=out[row0 : row0 + P, :], in_=o_sb)

            # ---------------- DMA emission (consumption order) --------------
            # The HBM stack is shared with the pair core and delivers as
            # little as ~150 KB/us during the all-cores-start burst, so
            # anything enqueued ahead of its need time steals bandwidth from
            # the critical path.  wk / h2 slice 0 gate the first projection
            # matmul and are split into ec-quarters (the tile dep tracker is
            # range-precise, so MMs on chunk 0 start ~4us earlier).
            cos_r = cos_t.rearrange("p (c s) -> p c s", s=S)
            sin_r = sin_t.rearrange("p (c s) -> p c s", s=S)
            wk_t = singles.tile([P, EC, D], bf16, tag="w_wk")
            wk_r = w_dram["wk"].rearrange("p (c d) -> p c d", d=D)
            for e in range(EC):
                nc.sync.dma_start(out=h2s[0][:, e : e + 1, :], in_=h2t_d[:, 0, e : e + 1])
                nc.scalar.dma_start(out=wk_t[:, e : e + 1, :], in_=wk_r[:, e : e + 1])
            w_sb["wk"] = wk_t
            bk_sb = singles.tile([P, DC], f32, tag="bk")
            nc.gpsimd.dma_start(out=bk_sb, in_=bk_c)
            # RoPE tables: first slice's worth now, the rest staged behind the
            # phase-A weights
            H = 2 * QW
            nc.gpsimd.dma_start(out=cos_sb[:, 0, 0:H], in_=cos_r[:, 0, 0:H])
            nc.scalar.dma_start(out=sin_sb[:, 0, 0:H], in_=sin_r[:, 0, 0:H])
            nc.gpsimd.dma_start(out=cos_sb[:, 1, 0:H], in_=cos_r[:, 1, 0:H])
            nc.scalar.dma_start(out=sin_sb[:, 1, 0:H], in_=sin_r[:, 1, 0:H])
            nc.sync.dma_start(out=h2s[1], in_=h2t_d[:, 1])
            w_sb["wq"] = load_w("wq", eng=nc.scalar)
            nc.scalar.dma_start(out=h1s[0], in_=h1t_d[:, 0])
            bq_sb = singles.tile([P, DC], f32, tag="bq")
            nc.gpsimd.dma_start(out=bq_sb, in_=bq_c)
            w_sb["wv"] = load_w("wv", eng=nc.gpsimd)
            nc.sync.dma_start(out=h2s[2], in_=h2t_d[:, 2])
            nc.gpsimd.dma_start(out=cos_sb[:, 0, H:S], in_=cos_r[:, 0, H:S])
            nc.scalar.dma_start(out=sin_sb[:, 0, H:S], in_=sin_r[:, 0, H:S])
            nc.gpsimd.dma_start(out=cos_sb[:, 1, H:S], in_=cos_r[:, 1, H:S])
            nc.scalar.dma_start(out=sin_sb[:, 1, H:S], in_=sin_r[:, 1, H:S])
            nc.sync.dma_start(out=h2s[3], in_=h2t_d[:, 3])
            ident = singles.tile([P, P], f32, tag="ident")
            nc.gpsimd.dma_start(out=ident, in_=ident_d)
            nc.scalar.dma_start(out=h1s[1], in_=h1t_d[:, 1])
            w_sb["wo"] = load_w("wo")
            bo_sb = singles.tile([P, D], f32, tag="bo")
            nc.gpsimd.dma_start(out=bo_sb, in_=bo_b)
            nc.scalar.dma_start(out=h1s[2], in_=h1t_d[:, 2])
            nc.scalar.dma_start(out=h1s[3], in_=h1t_d[:, 3])

            # ---------------- Phase A ++ q-tile 0 pass 1 --------------------
            # Interleave schedule matched to DMA arrival order (k0 chunk-
            # paced with junk filler; wv rides the gpsimd queue so v0 is
            # ready before q0's inputs land; kb blocks fill the rest).
            qs0 = new_qstate()
            junk_mms(NWARM)
            project_rope(h2s[0], "wk", bk_sb, kt_p[0], 0, junk=4)
            project_rope(h2s[1], "wk", bk_sb, kt_p[1], 1, junk=2)
            project_v(0)
            project_rope(h1s[0], "wq", bq_sb, qt_p[0], 0)
            for kb in range(0, 4):
                emit_kb_block(0, kb, qs0)
            project_v(1)
            for kb in range(4, 8):
                emit_kb_block(0, kb, qs0)
            project_rope(h2s[2], "wk", bk_sb, kt_p[2], 2)
            project_v(2)
            for kb in range(8, 12):
                emit_kb_block(0, kb, qs0)
            project_rope(h2s[3], "wk", bk_sb, kt_p[3], 3)
            project_v(3)
            # q tile 1's projection here: h1s[1] has landed, and the last 4
            # blocks + qt0's finish cover its serial STT/combine chain
            project_rope(h1s[1], "wq", bq_sb, qt_p[1], 1)
            for kb in range(12, 16):
                emit_kb_block(0, kb, qs0)

            # ---------------- attention tail ---------------------------------
            # the NEXT-next q projection is emitted mid-pass1 (kb 8), where
            # the DVE only runs light tree adds - its GpSimd combines finish
            # half a q tile before pass1(qt+1) needs them
            qstate = qs0
            for qt in range(QT):
                finish_qt(qt, qstate)
                if qt + 1 < QT:
                    qstate = new_qstate()
                    for kb in range(NB):
                        if kb == 8 and qt + 2 < QT:
                            project_rope(
                                h1s[qt + 2], "wq", bq_sb, qt_p[qt + 2], qt + 2
                            )
                        emit_kb_block(qt + 1, kb, qstate)

    nc.compile()
    return nc


def _get_compiled():
    global _compiled
    if _compiled is None:
        _compiled = _build()
    return _compiled


def _pack(x_t, nchunks):
    # [nchunks*P, S] -> [P, nchunks*S]: partition p holds chunks contiguously,
    # matching the SBUF tile layout exactly (max-size DMA packets)
    n = x_t.shape[1]
    return np.ascontiguousarray(
        x_t.reshape(nchunks, P, n).transpose(1, 0, 2).reshape(P, nchunks * n)
    )


def _host_tables():
    half = D // 2
    inv_freq = 1.0 / (10000.0 ** (np.arange(half, dtype=np.float32) / half))
    t = np.arange(S, dtype=np.float32)
    freqs = np.outer(t, inv_freq)
    emb = np.concatenate([freqs, freqs], axis=-1)  # [S, D]
    # the two d-halves of emb are identical - ship only [D/2, S] worth
    cos_t = _pack(np.cos(emb).T[: D // 2].astype(BF16), 2)
    sin_t = _pack(np.sin(emb).T[: D // 2].astype(BF16), 2)
    return cos_t, sin_t


def make_in_maps(**inputs):
    cos_t, sin_t = _host_tables()
    shared = {
        "cos_t": cos_t,
        "sin_t": sin_t,
        "wq_t": _pack(np.asarray(inputs["Wq"], np.float32).T.astype(BF16), EC),
        "wk_t": _pack(np.asarray(inputs["Wk"], np.float32).T.astype(BF16), EC),
        "wv_t": _pack(np.asarray(inputs["Wv"], np.float32).T.astype(BF16), EC),
        "wo_t": _pack(np.asarray(inputs["Wo"], np.float32).T.astype(BF16), EC),
        "bq_c": np.ascontiguousarray(np.asarray(inputs["bq"], np.float32).reshape(DC, P).T),
        "bk_c": np.ascontiguousarray(np.asarray(inputs["bk"], np.float32).reshape(DC, P).T),
        # bv contributes bv @ Wo.T to every output row - fold it into bo
        "bo_b": np.ascontiguousarray(
            np.broadcast_to(
                np.asarray(inputs["bo"], np.float32)
                + np.asarray(inputs["Wo"], np.float32)
                @ np.asarray(inputs["bv"], np.float32),
                (P, D),
            )
        ),
        "ident": np.eye(P, dtype=np.float32),
    }
    h1 = np.asarray(inputs["h1"], np.float32)
    h2 = np.asarray(inputs["h2"], np.float32)
    def _pack_h(h):
        # [S, D] -> [P, NS, EC, QW]: t[p, s2, ec, sq] = h[s2*QW+sq, ec*P+p]
        ht = h.T.astype(BF16)  # [D, S]
        return np.ascontiguousarray(
            ht.reshape(EC, P, NS, QW).transpose(1, 2, 0, 3)
        )

    return [
        dict(shared, h1t=_pack_h(h1[core]), h2t=_pack_h(h2[core]))
        for core in range(B)
    ]


def _install_ntff_hook():
    """The agent image's antenv lacks axon_hooks; rebuild the NTFF profile hook
    from libaxon_pjrt.so (mirrors trn_agent_boot._ntff_profile_via_ctypes)."""
    try:
        from antenv.axon_hooks import get_axon_ntff_profile_hook  # noqa: F401

        return
    except ImportError:
        pass
    import contextlib
    import ctypes
    import types

    so_path = "/opt/axon/libaxon_pjrt.so"
    try:
        lib = ctypes.CDLL(so_path)
    except OSError:
        return
    if not hasattr(lib, "axon_start_nrt_profile"):
        return
    lib.axon_start_nrt_profile.argtypes = [
        ctypes.POINTER(ctypes.c_int64),
        ctypes.c_size_t,
    ]
    lib.axon_start_nrt_profile.restype = ctypes.c_int64
    lib.axon_stop_nrt_profile.argtypes = [ctypes.c_char_p]
    lib.axon_stop_nrt_profile.restype = ctypes.c_int64

    @contextlib.contextmanager
    def _hook(output_dir, device_ids):
        import jax

        jax.devices()
        if device_ids:
            ids = (ctypes.c_int64 * len(device_ids))(*device_ids)
            rc = lib.axon_start_nrt_profile(ids, len(device_ids))
        else:
            rc = lib.axon_start_nrt_profile(None, 0)
        if rc != 0:
            raise RuntimeError(f"axon_start_nrt_profile rc={rc}")
        try:
            yield
        finally:
            n = lib.axon_stop_nrt_profile(str(output_dir).encode())
            print(f"ntff profile: {n} file(s) written to {output_dir}")

    import antenv

    mod = types.ModuleType("antenv.axon_hooks")
    mod.get_axon_ntff_profile_hook = lambda: _hook
    mod.set_axon_ntff_profile_hook = lambda h: None
    sys.modules["antenv.axon_hooks"] = mod
    antenv.axon_hooks = mod


def run(trace=False, tmpdir=None, trace_cores=None, **inputs):
    from concourse.bass_utils import run_bass_kernel_spmd

    if trace:
        _install_ntff_hook()
    nc = _get_compiled()
    in_maps = make_in_maps(**inputs)
    kwargs = {}
    if tmpdir is not None:
        kwargs["tmpdir"] = tmpdir
    if trace_cores is not None:
        kwargs["trace_cores"] = trace_cores
    res = run_bass_kernel_spmd(
        nc, in_maps, core_ids=list(range(B)), trace=trace, **kwargs
    )
    out = np.stack([res.results[i]["out"] for i in range(B)]).astype(np.float32)
    return out, res


def kernel(**inputs):
    out, _ = run(trace=False, **inputs)
    return out
